# revision 14
# baseline (speedup 1.0000x reference)
"""Adaptive thresholding (11x11 box mean, BORDER_REPLICATE, THRESH_BINARY_INV)
on 8 TRN2 NeuronCores, data-parallel over the batch dim (16 images/core).

v1 structure (DVE-scan-bound):
  - Host sends x as fp16, pre-arranged per image as [128 part, 4 seg, 533]
    with margins baked in: per segment cols 0..10 zero-head, 11..15 left
    edge-replicate, 16..527 the 512 pixels, 528..532 right edge-replicate.
    Partition q, segment p holds image row 128p + q.
  - ONE DVE sliding-window scan per image over the flat [128, 2121] view:
    state = (xp[t] + state) - xp[t-11]; the 11-col zero head drains state
    between segments.  Output W11 = horizontal 11-tap sums.
  - PE per image into one [128, 4, 512] fp32 PSUM tile (4 banks):
      bm_top/mid/bot  band matmuls on W11 (vertical 11-sum, edge weights
                      folded into top/bot), K=128 fp16
      idn             -121*I on x (the compare term), K=128 fp16
      bhn/bhp         cross-block halos as 32x32-tiled matmuls at
                      tile positions (0,96)/(96,0) so the two chains run
                      concurrently on disjoint PE sub-arrays.
    PSUM = W - 121*x.
  - ACT: Sign(PSUM - 242) -> fp8e4 {-1,0,+1}, one instruction per image
    spanning all 4 banks; DMA'd out as 1 byte/pixel.
  - Host: out = (sign bit clear) * 255.
"""
import sys
sys.path.insert(0, '/opt/trn_rl_repo')
import numpy as np
import concourse.bass as bass
import concourse.tile as tile
from concourse import bacc, mybir
from concourse.bass_utils import run_bass_kernel_spmd
from concourse.alu_op_type import AluOpType as ALU

F32 = mybir.dt.float32
F16 = mybir.dt.float16
F8 = mybir.dt.float8e4

N_CORES = 8
BATCH, H, W = 128, 512, 512
IMGS_PER_CORE = BATCH // N_CORES      # 16
BLK = 128
NBLK = H // BLK                       # 4
K = 11
PAD = K // 2                          # 5
ZH = K                                # zero head width
WT = ZH + PAD + W + PAD               # 533 segment width
X0 = ZH + PAD                         # x offset within segment (16)
FLAT = NBLK * WT                      # 2132
SCLEN = FLAT - ZH                     # 2121 scan steps


def _band_matrices(dtype=np.float16):
    r = np.arange(BLK)
    bm_mid = (np.abs(r[:, None] - r[None, :]) <= PAD).astype(dtype)
    bm_top = bm_mid.copy()
    for rr in range(PAD):
        bm_top[0, rr] += dtype(PAD - rr)
    bm_bot = bm_mid.copy()
    for rr in range(BLK - PAD, BLK):
        bm_bot[BLK - 1, rr] += dtype(rr - (BLK - PAD - 1))
    # halo next: operand partitions 0..4 (next seg rows 0..4 = image rows
    # 128(pos+1)+p), weights [5, 32] -> psum partitions 96..127:
    # contributes to out row r (rel c = r-96) iff r >= 123+p  <=>  c >= 27+p
    bhn = np.zeros((PAD, 32), dtype=dtype)
    for p in range(PAD):
        bhn[p, 27 + p:] = 1.0
    # halo prev: operand partitions 96..127 (prev seg rows 96..127, i.e.
    # image rows 128(pos-1)+96+k), weights at partitions 96..127 (must match
    # the tile row group) -> psum partitions 0..31: abs row 96+k contributes
    # to out row c iff 96+k >= 123 and c <= (96+k) - 123
    bhp = np.zeros((BLK, 32), dtype=dtype)
    for k in range(96 + 27, BLK):
        bhp[k, 0:k - 123 + 1] = 1.0
    idn = (-121.0 * np.eye(BLK)).astype(dtype)
    return {"bm_top": bm_top, "bm_mid": bm_mid, "bm_bot": bm_bot,
            "bhn": bhn, "bhp": bhp, "idn": idn}


def _build():
    nc = bacc.Bacc(None, target_bir_lowering=False, debug=False)
    x_d = nc.declare_dram_parameter("x", [IMGS_PER_CORE * BLK, FLAT], F16,
                                    isOutput=False)
    shapes = {"bm_top": [BLK, BLK], "bm_mid": [BLK, BLK], "bm_bot": [BLK, BLK],
              "bhn": [PAD, 32], "bhp": [BLK, 32], "idn": [BLK, BLK]}
    consts = {nm: nc.declare_dram_parameter(nm, sh, F16, isOutput=False)
              for nm, sh in shapes.items()}
    out_d = nc.declare_dram_parameter("out", [IMGS_PER_CORE * BLK, NBLK * W],
                                      F8, isOutput=True)
    xv = x_d[:].rearrange("(i q) c -> i q c", q=BLK)       # [16,128,2132]
    ov = out_d[:].rearrange("(i q) c -> i q c", q=BLK)     # [16,128,2048]

    with tile.TileContext(nc) as tc:
        with (
            tc.tile_pool(name="cpool", bufs=1) as cpool,
            tc.tile_pool(name="xin", bufs=5) as x_pool,
            tc.tile_pool(name="scr", bufs=5) as s_pool,
            tc.tile_pool(name="outp", bufs=3) as o_pool,
            tc.tile_pool(name="psum", bufs=2, space=bass.MemorySpace.PSUM) as ps_pool,
        ):
            ct = {}
            for nm, d in consts.items():
                t = cpool.tile(list(d.shape), d.dtype, tag=nm)
                # consts ride the otherwise-idle gpsimd (SWDGE) queue so the
                # first image DMAs are alone on the HWDGE queues
                nc.gpsimd.dma_start(t[:], d[:])
                ct[nm] = t
            bias_t = cpool.tile([BLK, 1], F32, tag="bias")
            nc.vector.memset(bias_t[:], -242.0)

            imgs = {}

            # PE warmup: ~4us of back-to-back matmuls while the first image
            # DMAs land, so HAM un-throttles the PE clock to 2.4 GHz; the
            # per-image matmul bursts afterwards keep gaps < 3.4us so it
            # stays warm.  Uses a scratch psum bank; operand is the (already
            # loaded or in-flight) idn const tile -- contents irrelevant.
            warm_ps = ps_pool.tile([BLK, NBLK, W], F32, tag="ps", name="warm")
            warm_src = cpool.tile([BLK, W], F16, tag="warmsrc")
            nc.vector.memset(warm_src[:], 1.0)
            for _ in range(18):
                nc.tensor.matmul(warm_ps[:, 0, :], ct["idn"][:], warm_src[:],
                                 start=True, stop=True)

            def front_img(i, per_seg=False):
                ximg = x_pool.tile([BLK, NBLK, WT], F16, tag="ximg")
                eng = nc.sync if i % 2 == 0 else nc.scalar
                flat = ximg[:].rearrange("q p c -> q (p c)")
                s = s_pool.tile([BLK, SCLEN], F16, tag="scr")
                if per_seg:
                    # split DMA + scan per segment so the first scan starts
                    # as soon as 1/4 of the image has landed (startup) and
                    # downstream matmuls unblock per segment (tail)
                    for pos in range(NBLK):
                        eng.dma_start(ximg[:, pos, :], xv[i][:, pos * WT:(pos + 1) * WT])
                        o0 = pos * WT
                        nc.vector.tensor_tensor_scan(
                            s[:, o0:o0 + WT - ZH],
                            flat[:, o0 + ZH:o0 + WT], flat[:, o0:o0 + WT - ZH],
                            0.0, op0=ALU.add, op1=ALU.subtract)
                else:
                    eng.dma_start(flat, xv[i])
                    nc.vector.tensor_tensor_scan(
                        s[:], flat[:, ZH:FLAT], flat[:, 0:SCLEN], 0.0,
                        op0=ALU.add, op1=ALU.subtract)
                imgs[i] = (ximg, s)

            def segof(pos):
                return pos * WT + (K - 1)

            def back_img_last(i):
                # per-segment-ready matmul order + per-bank ACT/DMA so the
                # final dependency chain is one segment deep
                ximg, s = imgs.pop(i)
                ps = ps_pool.tile([BLK, NBLK, W], F32, tag="ps", name=f"ps_{i}")
                oimg = o_pool.tile([BLK, NBLK, W], F8, tag="oimg")

                def bm(pos, stop=False):
                    sfx = "top" if pos == 0 else ("bot" if pos == NBLK - 1 else "mid")
                    nc.tensor.matmul(ps[:, pos, :], ct["bm_" + sfx][:],
                                     s[:, segof(pos):segof(pos) + W],
                                     start=True, stop=False)

                def idn(pos, stop=False):
                    nc.tensor.matmul(ps[:, pos, :], ct["idn"][:],
                                     ximg[:, pos, X0:X0 + W],
                                     start=False, stop=stop)

                def bhn(pos, stop=False):
                    nc.tensor.matmul(ps[96:128, pos, :], ct["bhn"][:],
                                     s[0:PAD, segof(pos + 1):segof(pos + 1) + W],
                                     start=False, stop=stop, tile_position=(0, 96))

                def bhp(pos, stop=False):
                    nc.tensor.matmul(ps[0:32, pos, :], ct["bhp"][96:128, :],
                                     s[96:128, segof(pos - 1):segof(pos - 1) + W],
                                     start=False, stop=stop, tile_position=(96, 0))

                def act_dma(pos):
                    nc.scalar.activation(oimg[:, pos, :], ps[:, pos, :],
                                         mybir.ActivationFunctionType.Sign,
                                         bias=bias_t[:], scale=1.0)
                    eng = nc.sync if pos % 2 == 0 else nc.scalar
                    eng.dma_start(ov[i][:, pos * W:(pos + 1) * W], oimg[:, pos, :])

                # bm (start=True) must be the first write into each bank
                bm(0); idn(0)                                   # seg0 ready
                bm(1); idn(1); bhp(1); bhn(0, stop=True)        # seg1: bank0 done
                act_dma(0)
                bm(2); idn(2); bhp(2); bhn(1, stop=True)        # seg2: bank1 done
                act_dma(1)
                bm(3); idn(3); bhp(3, stop=True); bhn(2, stop=True)  # seg3
                nc.scalar.activation(
                    oimg[:, 2:4, :].rearrange("q p c -> q (p c)"),
                    ps[:, 2:4, :].rearrange("q p c -> q (p c)"),
                    mybir.ActivationFunctionType.Sign,
                    bias=bias_t[:], scale=1.0)
                nc.sync.dma_start(ov[i][:, 2 * W:4 * W],
                                  oimg[:, 2:4, :].rearrange("q p c -> q (p c)"))

            def back_img(i):
                ximg, s = imgs.pop(i)
                ps = ps_pool.tile([BLK, NBLK, W], F32, tag="ps", name=f"ps_{i}")
                # full-array matmuls: band + idn per bank, grouped by weight
                # matrix so LDWEIGHTS amortizes
                for pos in range(NBLK):
                    sfx = "top" if pos == 0 else ("bot" if pos == NBLK - 1 else "mid")
                    nc.tensor.matmul(ps[:, pos, :], ct["bm_" + sfx][:],
                                     s[:, segof(pos):segof(pos) + W],
                                     start=True, stop=False)
                for pos in range(NBLK):
                    # bank 0 gets no bhp, bank 3 no bhn; stops set on last touch
                    nc.tensor.matmul(ps[:, pos, :], ct["idn"][:],
                                     ximg[:, pos, X0:X0 + W],
                                     start=False, stop=False)
                # halo chains: bhn tiles at (0, 96), bhp tiles at (96, 0);
                # the two chains run on disjoint PE sub-arrays
                for pos in range(NBLK - 1):
                    # bank 0's accumulation ends at its bhn (no bhp term)
                    nc.tensor.matmul(ps[96:128, pos, :], ct["bhn"][:],
                                     s[0:PAD, segof(pos + 1):segof(pos + 1) + W],
                                     start=False, stop=(pos == 0),
                                     tile_position=(0, 96))
                for pos in range(1, NBLK):
                    nc.tensor.matmul(ps[0:32, pos, :], ct["bhp"][96:128, :],
                                     s[96:128, segof(pos - 1):segof(pos - 1) + W],
                                     start=False, stop=True,
                                     tile_position=(96, 0))
                oimg = o_pool.tile([BLK, NBLK, W], F8, tag="oimg")
                nc.scalar.activation(
                    oimg[:].rearrange("q p c -> q (p c)"),
                    ps[:].rearrange("q p c -> q (p c)"),
                    mybir.ActivationFunctionType.Sign,
                    bias=bias_t[:], scale=1.0)
                eng = nc.sync if i % 2 == 0 else nc.scalar
                eng.dma_start(ov[i], oimg[:].rearrange("q p c -> q (p c)"))

            front_img(0, per_seg=True)
            front_img(1, per_seg=True)
            front_img(2)
            for i in range(IMGS_PER_CORE - 2):
                back_img(i)
                if i + 3 < IMGS_PER_CORE:
                    per_seg = (i + 3 >= IMGS_PER_CORE - 2)
                    front_img(i + 3, per_seg=per_seg)
                # keep the PE HAM-warm through each image's idle window so
                # the final backlog drains at the 2.4 GHz rate
                for _ in range(3):
                    nc.tensor.matmul(warm_ps[:, 0, :], ct["idn"][:],
                                     warm_src[:], start=True, stop=True)
            back_img_last(IMGS_PER_CORE - 2)
            back_img_last(IMGS_PER_CORE - 1)
    nc.compile()
    return nc


_NC_CACHE = None


def _make_in_maps(x: np.ndarray) -> list:
    x = np.asarray(x, dtype=np.float32)
    x16 = x.reshape(BATCH, H, W).astype(np.float16)
    consts = _band_matrices()
    in_maps = []
    for c in range(N_CORES):
        shard = x16[c * IMGS_PER_CORE:(c + 1) * IMGS_PER_CORE]
        # [16, H, W] -> [16, 128 part, 4 seg, 512]
        xs = shard.reshape(IMGS_PER_CORE, NBLK, BLK, W).transpose(0, 2, 1, 3)
        xp = np.zeros((IMGS_PER_CORE, BLK, NBLK, WT), dtype=np.float16)
        xp[:, :, :, X0:X0 + W] = xs
        xp[:, :, :, ZH:X0] = xs[:, :, :, 0:1]
        xp[:, :, :, X0 + W:] = xs[:, :, :, W - 1:W]
        m = {"x": np.ascontiguousarray(
            xp.reshape(IMGS_PER_CORE * BLK, FLAT))}
        m.update(consts)
        in_maps.append(m)
    return in_maps


def kernel(x: np.ndarray) -> np.ndarray:
    global _NC_CACHE
    if _NC_CACHE is None:
        _NC_CACHE = _build()
    nc = _NC_CACHE
    in_maps = _make_in_maps(x)
    res = run_bass_kernel_spmd(nc, in_maps, core_ids=list(range(N_CORES)))
    out = np.empty((BATCH, H, W), dtype=np.float32)
    for c in range(N_CORES):
        raw = res.results[c]["out"]  # [2048, 2048] fp8 -> view bytes
        b = raw.view(np.uint8).reshape(IMGS_PER_CORE, BLK, NBLK, W)
        # sign bit clear (>= 0) -> 255
        vals = np.where((b & 0x80) == 0, np.float32(255.0), np.float32(0.0))
        # [i, q, p, c] -> rows 128p + q
        out[c * IMGS_PER_CORE:(c + 1) * IMGS_PER_CORE] = \
            vals.transpose(0, 2, 1, 3).reshape(IMGS_PER_CORE, H, W)
    return out.reshape(BATCH, H, W, 1)
